# revision 1
# baseline (speedup 1.0000x reference)
"""CRF negative-log-likelihood loss kernel for Trainium2 (8 NeuronCores).

Strategy
--------
Data-parallel over the batch: 32 sequences -> 4 per core. Each core evaluates
the log-partition function in *linear* space with a bidirectional split that
halves the serial chain: the forward recurrence

    alpha_t = (M @ alpha_{t-1}) * e_t,      M = exp(T), e_t = exp(feat_t)

runs from t=0 up to t=255 while the backward recurrence

    beta_t  = M^T-contraction of (e_{t+1} * beta_{t+1}),   beta_511 = 1

runs from t=511 down to t=255; both are one 64x64 TensorEngine matmul plus
one elementwise VectorEngine multiply per step, and the two chains interleave
on the engines so the wall time is one chain's ~256-step latency. They meet
with  Z = sum_i alpha_255[i] * beta_255[i].

The matmul weights are augmented to [W | ONES] (bf16, single PE pass), so
every step's psum also delivers sum_i(state) broadcast across rows 64-127.
Every K=8 steps that sum renormalizes the chain: the reciprocal is folded
into a later step's emission operand (scale-invariance), so normalization
never touches the serial critical path. Each log-scale is evaluated as
Ln(s * 2^-48) (ScalarE Ln saturates at 2^64) and the 48*ln2 is added back at
the end.

The gold-path score (emissions at tags plus transitions) is computed with
one-hot matmuls: per sequence, G = [F | OH_next]^T @ OH_prev has feats^T@OH
in rows 0-63 (diagonal = emission score) and the transition-pair count matrix
in rows 64-127 (Frobenius product with T = transition score); one multiply
with [I; T], a row reduce, and a ones-matmul collapse it to scalars. All off
the critical path.

Host-side work is limited to input relayout: per-core slicing, one-hot
encoding of the integer tags (with a zero guard row), transposing T, and
concatenating eye(64) with T.
"""

import math

import numpy as np
from contextlib import ExitStack

B, T_LEN, L = 32, 512, 64
N_CORES = 8
BPC = B // N_CORES  # sequences per core
T_MID = 255         # chains meet here
K_NORM = 10         # renormalize every K steps
LN_SCALE = 2.0 ** -48

_compiled = None  # compiled program cache so repeated kernel() calls reuse it


def _build_program():
    import concourse.bacc as bacc
    import concourse.tile as tile
    import concourse.mybir as mybir
    from concourse.alu_op_type import AluOpType

    f32 = mybir.dt.float32
    bf16 = mybir.dt.bfloat16
    Af = mybir.ActivationFunctionType

    nc = bacc.Bacc("TRN2", target_bir_lowering=False, debug=False,
                   num_devices=N_CORES)

    # feats arrives t-major: row t*BPC+b holds feats[b, t, :]
    feats_d = nc.dram_tensor("feats", [BPC * T_LEN, L], f32,
                             kind="ExternalInput").ap()
    oh_d = nc.dram_tensor("oh", [BPC * (T_LEN + 1), L], f32,
                          kind="ExternalInput").ap()
    tt_d = nc.dram_tensor("tt", [L, L], f32, kind="ExternalInput").ap()
    mask_d = nc.dram_tensor("mask", [2 * L, L], f32, kind="ExternalInput").ap()
    out_d = nc.dram_tensor("out", [1, BPC], f32, kind="ExternalOutput").ap()

    with tile.TileContext(nc) as tc, ExitStack() as ctx:
        consts = ctx.enter_context(tc.tile_pool(name="consts", bufs=1))
        loadp = ctx.enter_context(tc.tile_pool(name="load", bufs=1))
        goldp = ctx.enter_context(tc.tile_pool(name="gold", bufs=16))
        alphap = ctx.enter_context(tc.tile_pool(name="alpha", bufs=4))
        vtmp = ctx.enter_context(tc.tile_pool(name="vtmp", bufs=6))
        qf = ctx.enter_context(tc.tile_pool(name="qfpsum", bufs=3, space="PSUM"))
        qb = ctx.enter_context(tc.tile_pool(name="qbpsum", bufs=3, space="PSUM"))
        tpp = ctx.enter_context(tc.tile_pool(name="tpsum", bufs=2, space="PSUM"))

        # ---- constants ----
        ones128 = consts.tile([128, 1], f32)
        nc.gpsimd.memset(ones128[:], 1.0)
        mask_sb = consts.tile([128, L], f32)
        nc.sync.dma_start(out=mask_sb[:], in_=mask_d)
        ttile = consts.tile([L, L], f32)          # T^T
        nc.sync.dma_start(out=ttile[:], in_=tt_d)
        tstr = consts.tile([L, L], f32)           # T (straight)
        nc.sync.dma_start(out=tstr[:], in_=mask_d[L:2 * L, :])
        # W3  = [exp(T)^T | ONES]  (forward);  W3b = [exp(T) | ONES] (backward)
        # Matmul against either gives the new state in psum rows 0-63 and the
        # input-state column sums broadcast across rows 64-127.
        W3 = consts.tile([L, 2 * L], bf16)
        nc.scalar.activation(W3[:, 0:L], ttile[:], Af.Exp)
        nc.gpsimd.memset(W3[:, L:2 * L], 1.0)
        W3b = consts.tile([L, 2 * L], bf16)
        nc.scalar.activation(W3b[:, 0:L], tstr[:], Af.Exp)
        nc.gpsimd.memset(W3b[:, L:2 * L], 1.0)

        # ---- e_feats in t-major chunks: efc[k][j, (t%32)*4 + b] (bf16) ----
        # Per chunk: contiguous-ish DMA of 128 t-major rows, Exp -> bf16 into
        # the left half of a [128,128] staging tile, then an xbar
        # DMA-transpose (2-byte dtype, free%128) whose partitions 0-63 are the
        # transposed chunk. No TensorEngine involvement, so the recurrence
        # matmuls never hit a PE tiling-mode switch. Chunks are emitted in the
        # order the two chains consume them (0, 15, 1, 14, ...).
        # Chunks are packed in pairs (w, 15-w) so one [128,128] xbar
        # transpose yields forward chunk w on partitions 0-63 and backward
        # chunk 15-w on partitions 64-127 (moved down by a small SBUF copy).
        # Window 0 already provides both chains' first chunks, so the
        # recurrence starts after one transpose. Wide Exps (4x [128,256])
        # replace 16 small ones; all copies precede all transposes to keep
        # DMA copy<->transpose mode transitions rare (Tile serializes them).
        packed = []
        for w in range(8):
            packed += [w, 15 - w]
        fcs, stgs = [None] * 4, [None] * 4
        eks, mvs = [None] * 8, [None] * 8
        last_mv = None

        def load_group(g):
            fcg = loadp.tile([128, 4 * L], f32, tag=f"fc{g}")
            for j in range(4):
                ck = packed[4 * g + j]
                nc.sync.dma_start(out=fcg[:, j * L:(j + 1) * L],
                                  in_=feats_d[ck * 128:(ck + 1) * 128, :])
            fcs[g] = fcg
            stg = loadp.tile([128, 4 * L], bf16, tag=f"stg{g}")
            nc.scalar.activation(stg[:], fcg[:], Af.Exp)
            stgs[g] = stg

        def transpose_window(w):
            nonlocal last_mv
            ek = consts.tile([128, 128], bf16, tag=f"ef{w}")
            nc.sync.dma_start(
                out=ek[:],
                in_=stgs[w // 2][:, (w % 2) * 128:(w % 2) * 128 + 128],
                transpose=True)
            eks[w] = ek
            mv = consts.tile([L, 128], bf16, tag=f"mv{w}")
            last_mv = nc.sync.dma_start(out=mv[:], in_=ek[L:128, :])
            mvs[w] = mv

        # group 0 end-to-end first: both chains' first chunks (0 and 15) are
        # ready after one transpose, so the recurrence starts ~10us earlier.
        load_group(0)
        transpose_window(0)
        transpose_window(1)
        for g in (1, 2, 3):
            load_group(g)
        for w in range(2, 8):
            transpose_window(w)

        def ef_col(t):  # [64, 4] AP of exp(feats[:, t, :]) for the 4 seqs
            k, col = t // 32, 4 * (t % 32)
            if k <= 7:
                return eks[k][0:L, col:col + 4]
            return mvs[15 - k][:, col:col + 4]

        # ---- bidirectional recurrence ----
        fwd_ev_tmp = set(range(K_NORM, T_MID - 2, K_NORM)) | {T_MID}
        bwd_ev_tmp = (set(range(T_LEN - 1 - K_NORM, T_MID + 3, -K_NORM))
                      | {T_MID + 1})
        n_events = len(fwd_ev_tmp) + len(bwd_ev_tmp) + 1
        lnS = consts.tile([1, 4 * n_events], f32)
        ev = 0

        def emit_ln(ps_row):  # ps_row: [1, BPC] psum AP holding s
            nonlocal ev
            nc.scalar.activation(lnS[:, 4 * ev:4 * ev + 4], ps_row,
                                 Af.Ln, scale=LN_SCALE)
            ev += 1

        alpha = alphap.tile([L, BPC], bf16, tag="alpha")
        nc.vector.tensor_copy(alpha[:], ef_col(0))
        v = alphap.tile([L, BPC], bf16, tag="v")
        nc.vector.tensor_copy(v[:], ef_col(T_LEN - 1))

        es_f = {}   # fwd step -> prescaled emission operand
        es_b = {}   # bwd step -> prescaled emission operand
        fwd_events = fwd_ev_tmp
        bwd_events = bwd_ev_tmp

        for s in range(T_MID):
            tf = 1 + s          # forward step index
            tb = T_LEN - 2 - s  # backward step index (mul at tb)

            # forward: q = W3^T @ alpha ; alpha = q[0:64] * e
            q = qf.tile([2 * L, BPC], f32, tag="q")
            nc.tensor.matmul(q[:], lhsT=W3[:], rhs=alpha[:],
                             start=True, stop=True)
            eop = es_f.pop(tf, None)
            if eop is None:
                eop = ef_col(tf)
            alpha_new = alphap.tile([L, BPC], bf16, tag="alpha")
            nc.vector.tensor_mul(alpha_new[:], q[0:L, :], eop)
            alpha = alpha_new
            if tf + 2 in fwd_events:  # 1/s(alpha_{tf-1}) lands at step tf+2
                rvf = vtmp.tile([L, BPC], f32, tag="rvf")
                nc.vector.reciprocal(rvf[:], q[L:2 * L, :])
                esf = vtmp.tile([L, BPC], f32, tag="esf")
                nc.gpsimd.tensor_mul(esf[:], ef_col(tf + 2), rvf[:])
                emit_ln(q[L:L + 1, :])
                es_f[tf + 2] = esf

            # backward: p = W3b^T @ v_{tb+1} ; v_tb = p[0:64] * e_tb
            p = qb.tile([2 * L, BPC], f32, tag="p")
            nc.tensor.matmul(p[:], lhsT=W3b[:], rhs=v[:],
                             start=True, stop=True)
            eop = es_b.pop(tb, None)
            if eop is None:
                eop = ef_col(tb)
            v_new = alphap.tile([L, BPC], bf16, tag="v")
            nc.vector.tensor_mul(v_new[:], p[0:L, :], eop)
            v = v_new
            if tb - 2 in bwd_events:
                rvb = vtmp.tile([L, BPC], f32, tag="rvb")
                nc.vector.reciprocal(rvb[:], p[L:2 * L, :])
                esb = vtmp.tile([L, BPC], f32, tag="esb")
                nc.gpsimd.tensor_mul(esb[:], ef_col(tb - 2), rvb[:])
                emit_ln(p[L:L + 1, :])
                es_b[tb - 2] = esb

        assert not es_f and not es_b, (sorted(es_f), sorted(es_b))
        # last backward contraction down to T_MID (no emission at T_MID here:
        # alpha_255 already carries e_255)
        p = qb.tile([2 * L, BPC], f32, tag="p")
        nc.tensor.matmul(p[:], lhsT=W3b[:], rhs=v[:], start=True, stop=True)

        # combine: Z_core = sum_i alpha_255[i] * beta_255[i]
        g = alphap.tile([L, BPC], bf16, tag="alpha")
        nc.vector.tensor_mul(g[:], p[0:L, :], alpha[:])
        qz = qf.tile([2 * L, BPC], f32, tag="q")
        qz_inst = nc.tensor.matmul(qz[:], lhsT=W3[:], rhs=g[:],
                                   start=True, stop=True)
        emit_ln(qz[L:L + 1, :])
        assert ev == n_events, ev

        fwd = vtmp.tile([1, BPC], f32, tag="fwd")
        nc.vector.tensor_reduce(
            fwd[:], lnS[:].rearrange("p (n b) -> p b n", b=BPC),
            axis=mybir.AxisListType.X, op=AluOpType.add)
        # add back the n_events * 48*ln2 removed by the Ln pre-scale
        lnoff = consts.tile([1, BPC], f32)
        nc.gpsimd.memset(lnoff[:], float(n_events * 48.0 * math.log(2.0)))
        fwd2 = vtmp.tile([1, BPC], f32, tag="fwd2")
        nc.vector.tensor_add(fwd2[:], fwd[:], lnoff[:])

        # ---- gold score via one-hot matmuls, forced after the loop ----
        # feats_d is t-major, so the F operand reads per-sequence strided
        # rows; oh stays (b t)-major with contiguous reads. Every gold matmul
        # gets an explicit dependency on the loop's final matmul: its
        # (128,128) PE tiling mode would otherwise interleave with the
        # (64,128) recurrence matmuls and each switch drains the PE.
        from concourse.tile_rust import add_dep_helper
        feats_bmaj = feats_d.rearrange("(t b) l -> b t l", b=BPC)
        Vt = consts.tile([128, BPC], f32)
        for b in range(BPC):
            gps = tpp.tile([128, L], f32, tag="tp")
            for c in range(4):
                o0 = b * (T_LEN + 1) + c * 128
                cat = goldp.tile([128, 128], f32, tag="cat")
                d1 = nc.sync.dma_start(
                    out=cat[:, 0:L],
                    in_=feats_bmaj[b, c * 128:(c + 1) * 128, :])
                d2 = nc.sync.dma_start(out=cat[:, L:2 * L],
                                       in_=oh_d[o0 + 1:o0 + 129, :])
                ohp = goldp.tile([128, L], f32, tag="ohp")
                d3 = nc.sync.dma_start(out=ohp[:], in_=oh_d[o0:o0 + 128, :])
                for dd in (d1, d2, d3):
                    add_dep_helper(dd.ins, last_mv.ins, sync=True,
                                   reason="gold copies after xbar transposes")
                gi = nc.tensor.matmul(gps[:], lhsT=cat[:], rhs=ohp[:],
                                      start=(c == 0), stop=(c == 3))
                add_dep_helper(gi.ins, qz_inst.ins, sync=True,
                               reason="gold matmuls after recurrence")
            gsc = vtmp.tile([128, L], f32, tag="gsc")
            nc.vector.tensor_mul(gsc[:], gps[:], mask_sb[:])
            nc.vector.tensor_reduce(Vt[:, b:b + 1], gsc[:],
                                    axis=mybir.AxisListType.X,
                                    op=AluOpType.add)
        gold_ps = tpp.tile([128, L], f32, tag="tp")
        nc.tensor.matmul(gold_ps[0:1, 0:BPC], lhsT=ones128[:, 0:1], rhs=Vt[:],
                         start=True, stop=True)

        res = vtmp.tile([1, BPC], f32, tag="res")
        nc.vector.tensor_tensor(res[:], fwd2[:], gold_ps[0:1, 0:BPC],
                                op=AluOpType.subtract)
        nc.sync.dma_start(out=out_d, in_=res[:])

    import concourse.bacc as bacc2
    orig = bacc2.Bacc.move_matmul_waits_to_ldweights
    if SKIP_LDW_WAIT_PASS:
        # Keep semaphore waits on the MATMUL itself so the (constant-weight)
        # LDWEIGHTS can issue while the previous step's DVE multiply runs.
        bacc2.Bacc.move_matmul_waits_to_ldweights = lambda self: None
    try:
        nc.compile()
    finally:
        bacc2.Bacc.move_matmul_waits_to_ldweights = orig
    return nc


SKIP_LDW_WAIT_PASS = True


def _prep_in_maps(feats, tags, T):
    feats = np.ascontiguousarray(np.asarray(feats, dtype=np.float32))
    T_np = np.ascontiguousarray(np.asarray(T, dtype=np.float32))
    tags_np = np.asarray(tags).astype(np.int64)

    oh = np.zeros((B, T_LEN + 1, L), dtype=np.float32)
    oh[np.arange(B)[:, None], np.arange(T_LEN)[None, :], tags_np] = 1.0
    mask_const = np.concatenate([np.eye(L, dtype=np.float32), T_np], axis=0)
    tt = np.ascontiguousarray(T_np.T)

    in_maps = []
    for c in range(N_CORES):
        sl = slice(c * BPC, (c + 1) * BPC)
        in_maps.append({
            "feats": np.ascontiguousarray(
                feats[sl].transpose(1, 0, 2).reshape(T_LEN * BPC, L)),
            "oh": np.ascontiguousarray(
                oh[sl].reshape(BPC * (T_LEN + 1), L)),
            "tt": tt,
            "mask": mask_const,
        })
    return in_maps


def kernel(feats, tags, T):
    global _compiled
    from concourse.bass_utils import run_bass_kernel_spmd

    if _compiled is None:
        _compiled = _build_program()
    nc = _compiled

    in_maps = _prep_in_maps(feats, tags, T)
    res = run_bass_kernel_spmd(nc, in_maps, list(range(N_CORES)))
    out = np.concatenate(
        [res.results[c]["out"].reshape(BPC) for c in range(N_CORES)])
    return out.astype(np.float32)



# revision 12
# speedup vs baseline: 1.2793x; 1.2793x over previous
"""CRF negative-log-likelihood loss kernel for Trainium2 (8 NeuronCores).

Strategy (v2: 4-way time-split, 128-round serial depth)
-------------------------------------------------------
Data-parallel over the batch: 32 sequences -> 4 per core. The log-partition
chain Z = 1^T A_511 ... A_1 e_0 (A_t = diag(e_t) M, M = exp(T)) is split into
four 128-step time chunks:

  - chunk [1,128):   forward vector chain  alpha_t = e_t * (M @ alpha_{t-1})
  - chunk [384,512): backward vector chain v_t = e_t * (M^T @ v_{t+1})
  - chunks [128,256) and [256,384): interior operators S = A_hi ... A_lo,
    computed as their transposes X = S^T by a basis chain
    Y <- e_t * (M^T @ Y) (t decreasing, Y seeded with diag(e_hi)), so that
    matmul(lhsT=X, rhs) applies S directly at combine time.

That cuts the serial dependency depth from 255 to ~128 rounds. Per round a
core runs 5 independent chains: one paired [alpha; v] vector chain ([128,4]
state, block-diag weights) and four interior stacks ([128,64] states holding
seq b's S1 chain on partitions 0-63 and its S2 chain on 64-127, block-diag
exp(T) weights). Interior per-step emission multiplies are per-partition
tensor_scalar ops spread across Vector, Scalar(Act) and GpSimd engines so no
engine exceeds the ~480ns hop latency.

All emissions are prescaled by 2^-7.5 (folded into the on-chip Exp bias);
magnitudes then stay inside fp32/bf16 range for the whole 512 steps, so the
kernel needs NO renormalization events. The host packs every emission operand
the chip will touch into one [128, 1028] fp32 stream (exp'd on-chip in 4
chunks); there are no on-chip transposes.

The gold-path score reuses the one-hot matmul scheme of v1, forced after the
recurrence.
"""

import math

import numpy as np
from contextlib import ExitStack

B, T_LEN, L = 32, 512, 64
N_CORES = 8
BPC = B // N_CORES  # sequences per core
R = 128             # rounds (chunk length)
PRE_BITS = 7.5      # emission prescale 2^-PRE_BITS, folded into Exp bias
NCOL = 4 + 8 * R    # seed cols + 8 cols per round

_compiled = None


def _build_program():
    import concourse.bacc as bacc
    import concourse.tile as tile
    import concourse.mybir as mybir
    from concourse.alu_op_type import AluOpType

    f32 = mybir.dt.float32
    bf16 = mybir.dt.bfloat16
    Af = mybir.ActivationFunctionType

    nc = bacc.Bacc("TRN2", target_bir_lowering=False, debug=False,
                   num_devices=N_CORES)

    eops_d = nc.dram_tensor("eops", [128, NCOL], f32,
                            kind="ExternalInput").ap()
    feats_d = nc.dram_tensor("feats", [BPC * T_LEN, L], f32,
                             kind="ExternalInput").ap()
    oh_d = nc.dram_tensor("oh", [BPC * (T_LEN + 1), L], f32,
                          kind="ExternalInput").ap()
    tt_d = nc.dram_tensor("tt", [L, L], f32, kind="ExternalInput").ap()
    mask_d = nc.dram_tensor("mask", [2 * L, L], f32, kind="ExternalInput").ap()
    i2_d = nc.dram_tensor("i2", [128, L], f32, kind="ExternalInput").ap()
    out_d = nc.dram_tensor("out", [1, BPC], f32, kind="ExternalOutput").ap()

    EXP_BIAS = float(-PRE_BITS * math.log(2.0))
    LN_OFF = float(T_LEN * PRE_BITS * math.log(2.0))

    with tile.TileContext(nc) as tc, ExitStack() as ctx:
        consts = ctx.enter_context(tc.tile_pool(name="consts", bufs=1))
        goldp = ctx.enter_context(tc.tile_pool(name="gold", bufs=16))
        vstate = ctx.enter_context(tc.tile_pool(name="vstate", bufs=3))
        ypools = [ctx.enter_context(tc.tile_pool(name=f"y{s}", bufs=3))
                  for s in range(4)]
        vtmp = ctx.enter_context(tc.tile_pool(name="vtmp", bufs=8))
        vq = ctx.enter_context(tc.tile_pool(name="vq", bufs=1, space="PSUM"))
        pps = [ctx.enter_context(tc.tile_pool(name=f"p{s}", bufs=1,
                                              space="PSUM"))
               for s in range(4)]
        cps = ctx.enter_context(tc.tile_pool(name="cps", bufs=2, space="PSUM"))
        tpp = ctx.enter_context(tc.tile_pool(name="tpsum", bufs=1,
                                             space="PSUM"))

        # ---- constants ----
        ones128 = consts.tile([128, 1], f32)
        nc.gpsimd.memset(ones128[:], 1.0)
        onesb = consts.tile([L, 1], bf16)
        nc.gpsimd.memset(onesb[:], 1.0)
        mask_sb = consts.tile([128, L], f32)
        nc.sync.dma_start(out=mask_sb[:], in_=mask_d)
        ttile = consts.tile([L, L], f32)          # T^T
        nc.sync.dma_start(out=ttile[:], in_=tt_d)
        tstr = consts.tile([L, L], f32)           # T (straight)
        nc.sync.dma_start(out=tstr[:], in_=mask_d[L:2 * L, :])
        i2f = consts.tile([128, L], f32)          # [I64; I64]
        nc.sync.dma_start(out=i2f[:], in_=i2_d)
        ebias = consts.tile([128, 1], f32)
        nc.gpsimd.memset(ebias[:], EXP_BIAS)

        # Wc: paired vector-chain weights. cols 0-63 = [exp(T^T); 0]
        # (fwd M-contraction), cols 64-127 = [0; exp(T)] (bwd M^T).
        Wc = consts.tile([128, 128], bf16)
        nc.gpsimd.memset(Wc[:], 0.0)
        nc.scalar.activation(Wc[0:L, 0:L], ttile[:], Af.Exp)
        nc.scalar.activation(Wc[L:128, L:128], tstr[:], Af.Exp)
        # Wd: interior stack weights, block-diag(exp(T), exp(T)): both halves
        # perform the M^T contraction out = M^T @ Y.
        Wd = consts.tile([128, 128], bf16)
        nc.gpsimd.memset(Wd[:], 0.0)
        nc.scalar.activation(Wd[0:L, 0:L], tstr[:], Af.Exp)
        nc.scalar.activation(Wd[L:128, L:128], tstr[:], Af.Exp)

        # ---- emission stream: DMA fp32, Exp(+bias) in 4 chunks ----
        # cols [0,516) = seeds + vector-chain operands -> bf16 (matmul/TT);
        # cols [516,1028) = interior ts scalars -> fp32 (tensor_scalar
        # requires fp32 scalar operands).
        eall_v = consts.tile([128, 516], bf16)
        eall_s = consts.tile([128, 512], f32)
        for k, (a, bnd) in enumerate([(0, 260), (260, 516),
                                      (516, 772), (772, NCOL)]):
            stg = consts.tile([128, bnd - a], f32, tag=f"estg{k}")
            nc.sync.dma_start(out=stg[:], in_=eops_d[:, a:bnd])
            dst = (eall_v[:, a:bnd] if bnd <= 516
                   else eall_s[:, a - 516:bnd - 516])
            nc.scalar.activation(dst, stg[:], Af.Exp, bias=ebias[:])

        def veccol(r):
            return eall_v[:, 4 + 4 * r: 8 + 4 * r]

        def stackcol(r, s):
            c = 4 * r + s
            return eall_s[:, c:c + 1]

        # ---- 128 rounds: 1 paired vector chain + 4 interior stacks ----
        # ts engine assignment: stack0 -> DVE, stacks 1,2 -> Act, stack3 ->
        # GpSimd; the vector TT also rides DVE.
        state = eall_v[:, 0:4]  # seeds [alpha_0; v_511]
        Y = [None] * 4
        for s in range(4):
            y0 = ypools[s].tile([128, L], bf16, tag=f"y{s}")
            if s == 0:
                nc.vector.tensor_scalar_mul(y0[:], i2f[:], stackcol(0, s))
            elif s == 3:
                nc.gpsimd.tensor_scalar_mul(y0[:], i2f[:], stackcol(0, s))
            else:
                nc.scalar.activation(y0[:], i2f[:], Af.Copy,
                                     scale=stackcol(0, s))
            Y[s] = y0

        last_mm = None
        for r in range(1, R + 1):
            # vector chain: rounds 1..127 consume e_{r} / e_{511-r}
            if r <= R - 1:
                q = vq.tile([128, 4], f32, tag="q")
                nc.tensor.matmul(q[:], lhsT=Wc[:], rhs=state,
                                 start=True, stop=True)
                ns = vstate.tile([128, 4], bf16, tag="vs")
                nc.vector.tensor_tensor(ns[:], q[:], veccol(r - 1),
                                        op=AluOpType.mult)
                state = ns[:]
            for s in range(4):
                p = pps[s].tile([128, L], f32, tag=f"p{s}")
                mi = nc.tensor.matmul(p[:], lhsT=Wd[:], rhs=Y[s][:],
                                      start=True, stop=True)
                last_mm = mi
                if r <= R - 1:
                    yn = ypools[s].tile([128, L], bf16, tag=f"y{s}")
                    if s in (0, 3):
                        nc.vector.tensor_scalar_mul(yn[:], p[:],
                                                    stackcol(r, s))
                    else:
                        nc.scalar.activation(yn[:], p[:], Af.Copy,
                                             scale=stackcol(r, s))
                    Y[s] = yn
                else:
                    # r == R: final X = M^T Y_127 -> SBUF bf16
                    xs = vtmp.tile([128, L], bf16, tag=f"x{s}")
                    nc.vector.tensor_copy(xs[:], p[:])
                    Y[s] = xs

        # ---- combine ----
        # partition-aligned copies of the bottom halves (X2 of each stack,
        # and v from the vector state) via SBUF->SBUF DMA
        X2 = []
        for s in range(4):
            x2 = vtmp.tile([L, L], bf16, tag=f"x2{s}")
            nc.sync.dma_start(out=x2[:], in_=Y[s][L:128, :])
            X2.append(x2)
        vlow = vtmp.tile([L, 4], bf16, tag="vlow")
        nc.sync.dma_start(out=vlow[:], in_=state[L:128, 0:4])

        # u = M^T v_384  (batched over the 4 seqs)
        ups = cps.tile([L, 4], f32, tag="c")
        nc.tensor.matmul(ups[:], lhsT=Wd[0:L, 0:L], rhs=vlow[:],
                         start=True, stop=True)
        usb = vtmp.tile([L, 4], bf16, tag="usb")
        nc.vector.tensor_copy(usb[:], ups[:])

        g = vtmp.tile([L, 4], bf16, tag="g")
        for b in range(4):
            z1 = cps.tile([L, 1], f32, tag="c")
            nc.tensor.matmul(z1[:], lhsT=Y[b][0:L, :], rhs=state[0:L, b:b + 1],
                             start=True, stop=True)
            z1s = vtmp.tile([L, 1], bf16, tag=f"z1s{b}")
            nc.vector.tensor_copy(z1s[:], z1[:])
            z2 = cps.tile([L, 1], f32, tag="c")
            nc.tensor.matmul(z2[:], lhsT=X2[b][:], rhs=z1s[:],
                             start=True, stop=True)
            nc.vector.tensor_mul(g[:, b:b + 1], z2[:], usb[:, b:b + 1])
        zrow = cps.tile([1, 4], f32, tag="c")
        zi = nc.tensor.matmul(zrow[:], lhsT=onesb[:], rhs=g[:],
                              start=True, stop=True)
        lnz = vtmp.tile([1, 4], f32, tag="lnz")
        nc.scalar.activation(lnz[:], zrow[:], Af.Ln)

        # ---- gold score via one-hot matmuls, forced after the loop ----
        from concourse.tile_rust import add_dep_helper
        feats_bmaj = feats_d.rearrange("(b t) l -> b t l", b=BPC)
        Vt = consts.tile([128, BPC], f32)
        for b in range(BPC):
            gps = tpp.tile([128, L], f32, tag="tp")
            for c in range(4):
                o0 = b * (T_LEN + 1) + c * 128
                cat = goldp.tile([128, 128], f32, tag="cat")
                nc.sync.dma_start(
                    out=cat[:, 0:L],
                    in_=feats_bmaj[b, c * 128:(c + 1) * 128, :])
                nc.sync.dma_start(out=cat[:, L:2 * L],
                                  in_=oh_d[o0 + 1:o0 + 129, :])
                ohp = goldp.tile([128, L], f32, tag="ohp")
                nc.sync.dma_start(out=ohp[:], in_=oh_d[o0:o0 + 128, :])
                gi = nc.tensor.matmul(gps[:], lhsT=cat[:], rhs=ohp[:],
                                      start=(c == 0), stop=(c == 3))
                add_dep_helper(gi.ins, zi.ins, sync=True,
                               reason="gold matmuls after combine")
            gsc = vtmp.tile([128, L], f32, tag="gsc")
            nc.vector.tensor_mul(gsc[:], gps[:], mask_sb[:])
            nc.vector.tensor_reduce(Vt[:, b:b + 1], gsc[:],
                                    axis=mybir.AxisListType.X,
                                    op=AluOpType.add)
        gold_ps = tpp.tile([128, L], f32, tag="tp")
        nc.tensor.matmul(gold_ps[0:1, 0:BPC], lhsT=ones128[:, 0:1], rhs=Vt[:],
                         start=True, stop=True)

        res0 = vtmp.tile([1, BPC], f32, tag="res0")
        nc.vector.tensor_tensor(res0[:], lnz[:], gold_ps[0:1, 0:BPC],
                                op=AluOpType.subtract)
        res = vtmp.tile([1, BPC], f32, tag="res")
        nc.vector.tensor_scalar_add(res[:], res0[:], LN_OFF)
        nc.sync.dma_start(out=out_d, in_=res[:])

    import concourse.bacc as bacc2
    orig = bacc2.Bacc.move_matmul_waits_to_ldweights
    if SKIP_LDW_WAIT_PASS:
        bacc2.Bacc.move_matmul_waits_to_ldweights = lambda self: None
    try:
        nc.compile()
    finally:
        bacc2.Bacc.move_matmul_waits_to_ldweights = orig
    return nc


SKIP_LDW_WAIT_PASS = True


def _prep_in_maps(feats, tags, T):
    feats = np.ascontiguousarray(np.asarray(feats, dtype=np.float32))
    T_np = np.ascontiguousarray(np.asarray(T, dtype=np.float32))
    tags_np = np.asarray(tags).astype(np.int64)

    oh = np.zeros((B, T_LEN + 1, L), dtype=np.float32)
    oh[np.arange(B)[:, None], np.arange(T_LEN)[None, :], tags_np] = 1.0
    mask_const = np.concatenate([np.eye(L, dtype=np.float32), T_np], axis=0)
    tt = np.ascontiguousarray(T_np.T)
    i2 = np.concatenate([np.eye(L, dtype=np.float32)] * 2, axis=0)

    r7 = np.arange(R - 1)
    r8 = np.arange(R)
    in_maps = []
    for c in range(N_CORES):
        sl = slice(c * BPC, (c + 1) * BPC)
        fb = feats[sl]  # [4, 512, 64]
        eops = np.zeros((128, NCOL), dtype=np.float32)
        for b in range(BPC):
            eops[0:L, b] = fb[b, 0]
            eops[L:128, b] = fb[b, T_LEN - 1]
            eops[0:L, 4 + 4 * r7 + b] = fb[b, 1 + r7].T
            eops[L:128, 4 + 4 * r7 + b] = fb[b, T_LEN - 2 - r7].T
            eops[0:L, 516 + 4 * r8 + b] = fb[b, 2 * R - 1 - r8].T
            eops[L:128, 516 + 4 * r8 + b] = fb[b, 3 * R - 1 - r8].T
        in_maps.append({
            "eops": eops,
            "feats": np.ascontiguousarray(fb.reshape(BPC * T_LEN, L)),
            "oh": np.ascontiguousarray(oh[sl].reshape(BPC * (T_LEN + 1), L)),
            "tt": tt,
            "mask": mask_const,
            "i2": i2,
        })
    return in_maps


def kernel(feats, tags, T):
    global _compiled
    from concourse.bass_utils import run_bass_kernel_spmd

    if _compiled is None:
        _compiled = _build_program()
    nc = _compiled

    in_maps = _prep_in_maps(feats, tags, T)
    res = run_bass_kernel_spmd(nc, in_maps, list(range(N_CORES)))
    out = np.concatenate(
        [res.results[c]["out"].reshape(BPC) for c in range(N_CORES)])
    return out.astype(np.float32)
